# revision 32
# baseline (speedup 1.0000x reference)
"""Trainium2 Bass kernel for the OFPenalty eigenvalue-penalty loss.

Math (per sample b of 256):
  W = x[b] reshaped [C=2048, N=49];  G = W^T W  (49x49 Gram matrix)
  run1: x9 = G^9 x0 (normalization deferred - scale invariant),
        largest = Rayleigh(G, x9) = (G x9 . x9) / (x9 . x9)
  run2: B = G - largest*I, u9 = B^9 x1 (x1 = scaled x9),
        tmp = Rayleigh(B, u9); smallest = tmp + largest
  penalty = (largest/smallest - 1)^2 = (tmp/smallest)^2; output = mean.

Implementation notes:
  - Pure data parallel: 32 samples per core on 8 cores.  Samples are
    processed in pairs packed along partitions: sample 2p in rows 0:64
    (real 0:49), sample 2p+1 in rows 64:128 (real 64:113).  All square
    matrices use a PACKED [128, 64] layout (block b occupies rows
    64b:64b+49, cols 0:49 of its half) so every PSUM->SBUF move is one
    [128, 64] copy.
  - Gram matmuls run in bf16 (1 PE cycle/row vs 4 for fp32); x tiles are
    converted fp32->bf16 once on ACT/Pool.  Stationary and moving tiles
    are zero-padded to 64 columns so all pad rows/cols of every product
    stay exactly 0 - no masks needed anywhere downstream.
  - G^9 x0 is evaluated log-depth: G2=G^2, G4=G2^2, G8=G4^2 (bf16
    squarings, fp32 PSUM accumulate), then x9 = G8*(G*x0), w1 = A*x9.
    Run2: Ball = A - lam*I (built on DVE from Abf + rank-1 broadcast
    lamI), B2/B4/B8 squarings, u1 = S102*(w1 - lam*x9) directly from
    run1 PSUMs (no extra matvec), u9 = B8*u1, w2 = Ball*u9.
  - Rayleigh quotients: per-sample columns T = (v*S104)*w on DVE read
    the matvec PSUMs directly, a ones-vector matmul reduces over
    partitions, scalar chain on free-dim rows.  Scalings by powers of
    two (exact) keep all intermediates inside fp32 range.
  - Pairs stream one at a time behind the (serialized, 360 B/ns) DMA
    queue; Rayleigh/shift work is batched in groups [6,6,3,1] so only
    the last small group's chain is exposed after the final DMA.
"""

import os
import sys
from contextlib import ExitStack

import numpy as np

for _p in ("/opt/trn_rl_repo",):
    if os.path.isdir(_p) and _p not in sys.path:
        sys.path.insert(0, _p)

import concourse.bass as bass  # noqa: E402
import concourse.tile as tile  # noqa: E402
from concourse import bacc, mybir  # noqa: E402
from concourse.bass_utils import run_bass_kernel_spmd  # noqa: E402

F32 = mybir.dt.float32
BF16 = mybir.dt.bfloat16
ALU = mybir.AluOpType

B, C, N = 256, 2048, 49
NCORES = 8
BS = B // NCORES  # 32 samples per core
NPAIR = BS // 2  # 16 pairs
KT = C // 128  # 16 contraction tiles
PG = 128
B1 = 64  # partition base of the second sample in a pair
PD = 64  # padded block width (cols 49:64 always zero)
S52 = float(2.0**-52)
S104 = float(2.0**-104)  # scale one side of Rayleigh products
S102 = float(2.0**-102)  # rescale x9 -> x1 (run2 warm start)
GROUPS = [3, 3, 3, 3, 2, 2]  # pairs per Rayleigh group (sum = NPAIR)
GWMAX = max(GROUPS)


def _emit(tc, x, x0, pen, repeat=1, dbg=None, dbg2=None):
    nc = tc.nc
    ctx = ExitStack()
    with ctx:
        const = ctx.enter_context(tc.tile_pool(name="const", bufs=1))
        xpool = ctx.enter_context(tc.tile_pool(name="xt", bufs=5))
        psp = ctx.enter_context(tc.tile_pool(name="psp", bufs=1, space="PSUM"))
        # PSUM dependency tracking is bank-granular: a read of any part of
        # a bank serializes against the next write to that bank.  Assign
        # banks so bank-order matches true dependency order:
        #   GRB[p%2]: pair p's Gram accumulation + y matvec
        #   SQ1[p%2]: pair p's G2/G4/G8 squaring chain (truly serial)
        #   SQ2[j%2]: run2 B2/B4/B8 chains per group lane
        #   MVB[g%2]: group g's matvec columns, nd rows, psl (sequential)
        GRB = [psp.tile([PG, 512], F32, name=f"GRB{i}") for i in range(2)]
        SQ1 = [psp.tile([PG, 512], F32, name=f"SQ1_{i}") for i in range(2)]
        SQ2 = [psp.tile([PG, 512], F32, name=f"SQ2_{i}") for i in range(2)]
        MVB = [psp.tile([PG, 512], F32, name=f"MVB{i}") for i in range(2)]

        class BankSlots:
            def __init__(self, banks, width):
                self.banks, self.width = banks, width
                self.per = 512 // width
                self.cnt = [0] * len(banks)
            def get(self, lane):
                b = lane % len(self.banks)
                s = self.cnt[b] % self.per
                self.cnt[b] += 1
                return self.banks[b][:, s * self.width : (s + 1) * self.width]

        gram_slots = BankSlots(GRB, PD)
        gram_slots.per = 7  # columns 448+ reserved for y matvec slots
        sq1_slots = BankSlots(SQ1, PD)
        sq2_slots = BankSlots(SQ2, PD)

        # ---- constants -------------------------------------------------
        # x0 columns: X0[0:49, p] = x0[2p], X0[64:113, p] = x0[2p+1]
        X0F = const.tile([PG, NPAIR], F32)
        nc.gpsimd.memset(X0F[:], 0.0)
        x0r = x0.rearrange("(p two) j -> two j p", two=2)
        nc.sync.dma_start(X0F[0:N, :], x0r[0])
        nc.sync.dma_start(X0F[B1 : B1 + N, :], x0r[1])

        # packed identity: DIAG[q, j] = 1 iff (q % 64) == j
        DIAG = const.tile([PG, PD], F32)
        nc.gpsimd.memset(DIAG[:], 0.0)
        for blk in range(2):
            nc.gpsimd.affine_select(
                out=DIAG[blk * B1 : (blk + 1) * B1, :],
                in_=DIAG[blk * B1 : (blk + 1) * B1, :],
                compare_op=ALU.not_equal,
                fill=1.0,
                base=0,
                pattern=[[-1, PD]],
                channel_multiplier=1,
            )

        # CMS row s: 1 on the partitions sample s of a pair owns (for the
        # rank-1 lambda broadcast); ONE2 col s: ones on block s partitions
        # (two-half partition reduction in one matmul).
        CMS = const.tile([2, PG], F32)
        CM1 = const.tile([1, PG], F32)
        nc.gpsimd.memset(CMS[:], 0.0)
        nc.gpsimd.memset(CMS[0:1, 0:N], 1.0)
        nc.gpsimd.memset(CM1[:], 0.0)
        nc.gpsimd.memset(CM1[:, B1 : B1 + N], 1.0)
        # memset can't target partition base 1; place row 1 via sbuf dma
        nc.sync.dma_start(CMS[1:2, :], CM1[:])
        ONE2 = const.tile([PG, 2], F32)
        nc.gpsimd.memset(ONE2[:], 0.0)
        nc.gpsimd.memset(ONE2[0:B1, 0:1], 1.0)
        nc.gpsimd.memset(ONE2[B1:PG, 1:2], 1.0)

        # ---- persistent state ------------------------------------------
        NXB = 5  # bf16 x-tile slots (pads memset once)
        xb = []
        for i in range(NXB):
            t = const.tile([PG, 2, KT, PD], BF16, name=f"xb{i}")
            nc.gpsimd.memset(t[:, :, :, N:PD], 0.0)
            xb.append(t)

        # one tile per pair: no ring reuse, no WAR hazards anywhere.
        # Everything after the Gram is fp32: the second power iteration is
        # an UNCONVERGED mixture whose value sits at the fp32 rounding
        # noise floor of the reference; bf16 noise in any matrix/vector of
        # the chain gets amplified ~(3943/2000)^9 and converges it toward
        # the true lambda_min, wrecking the match (hw err 2e2 vs 2e-4).
        Amat = [const.tile([PG, PD], F32, name=f"Amat{i}") for i in range(NPAIR)]
        G2m = [const.tile([PG, PD], F32, name=f"G2m{i}") for i in range(NPAIR)]
        G4m = [const.tile([PG, PD], F32, name=f"G4m{i}") for i in range(NPAIR)]
        Ballm = [const.tile([PG, PD], F32, name=f"Ballm{i}") for i in range(NPAIR)]
        B2m = [const.tile([PG, PD], F32, name=f"B2m{i}") for i in range(NPAIR)]
        B4m = [const.tile([PG, PD], F32, name=f"B4m{i}") for i in range(NPAIR)]
        B8m = [const.tile([PG, PD], F32, name=f"B8m{i}") for i in range(NPAIR)]
        SCR = [const.tile([PG, PD + 4], F32, name=f"SCR{i}") for i in range(NPAIR)]

        Y = const.tile([PG, NPAIR], F32)  # y = A*x0 columns
        Z = const.tile([PG, NPAIR], F32)  # z = G4*y columns
        X9 = const.tile([PG, NPAIR], F32)
        U1 = const.tile([PG, NPAIR], F32)  # run2 warm starts
        U9 = const.tile([PG, NPAIR], F32)
        LAMV = const.tile([PG, NPAIR], F32)  # per-partition lambda columns
        T1 = const.tile([PG, 2 * NPAIR], F32)  # Rayleigh-1 cols [num | den]
        T2 = const.tile([PG, 2 * NPAIR], F32)  # Rayleigh-2 cols
        LAM = const.tile([2, NPAIR], F32)  # LAM[s, p] = lambda(sample 2p+s)
        RDr = const.tile([2, NPAIR], F32)
        TMPr = const.tile([2, NPAIR], F32)
        SMr = const.tile([2, NPAIR], F32)
        RSr = const.tile([2, NPAIR], F32)
        RTr = const.tile([2, NPAIR], F32)
        PEN = const.tile([2, NPAIR], F32)  # PEN[s, p] = penalty(sample 2p+s)

        for _rep in range(repeat):
            # DMA layout: partition q holds c-rows {512b + 4q + r : r<4};
            # 784B-contiguous descriptors (>=512B keeps DMA at full rate).
            xrs = x.rearrange(
                "(p two) (b q r) j -> p two q b (r j)", two=2, b=4, q=128, r=4
            )

            pair_group = []
            for g, gw in enumerate(GROUPS):
                pair_group += [g] * gw
            group_base = [sum(GROUPS[:g]) for g in range(len(GROUPS))]
            group_end = [group_base[g] + GROUPS[g] - 1 for g in range(len(GROUPS))]

            # per-group column layout inside MVB[g%2]
            y_ps = [GRB[p % 2][:, 500 + (p // 2) % 3 : 501 + (p // 2) % 3]
                    for p in range(NPAIR)]
            mv1_ps, mv2_ps, nd1_ps, nd2_ps, psl_ps, wv_ps = [], [], [], [], [], []
            for g, gw in enumerate(GROUPS):
                bank = MVB[g % 2]
                mv1_ps.append(bank[:, 0 : 3 * gw])
                mv2_ps.append(bank[:, 3 * gw : 5 * gw])
                nd1_ps.append(bank[0:2, 5 * gw : 7 * gw])
                nd2_ps.append(bank[0:2, 7 * gw : 9 * gw])
                psl_ps.append(bank[:, 9 * gw : 10 * gw])
                wv_ps.append((bank[:, 10 * gw : 11 * gw], bank[:, 11 * gw : 12 * gw]))

            def matvec(ps_out, col, stat, mov_col, accum=False):
                for blk in range(2):
                    r0 = blk * B1
                    nc.tensor.matmul(
                        ps_out[r0 : r0 + B1, col : col + 1],
                        stat[r0 : r0 + B1, :],
                        mov_col[r0 : r0 + B1, :],
                        start=not accum,
                        stop=True,
                    )

            def sq_mm(src_bf, slots, lane):
                ps = slots.get(lane)
                for blk in range(2):
                    r0 = blk * B1
                    nc.tensor.matmul(
                        ps[r0 : r0 + B1, :],
                        src_bf[r0 : r0 + B1, :],
                        src_bf[r0 : r0 + B1, :],
                        start=True,
                        stop=True,
                    )
                return ps

            def sq_copy(dst_bf, ps, eng):
                if eng is nc.scalar:
                    eng.copy(dst_bf[:], ps[:])
                else:
                    eng.tensor_copy(dst_bf[:], ps[:])

            def ray_cols(p, m, gw, j, T):
                # T num col p = (w*S104)*v, den col NPAIR+p = (v*S104)*v
                vc = m[:, j : j + 1]
                wc = m[:, gw + j : gw + j + 1]
                scr = SCR[p]
                tw = scr[:, 0:1]
                tx = scr[:, 1:2]
                nc.vector.tensor_scalar(tw, wc, S104, None, op0=ALU.mult)
                nc.vector.tensor_scalar(tx, vc, S104, None, op0=ALU.mult)
                nc.vector.tensor_mul(T[:, p : p + 1], tw, vc)
                nc.vector.tensor_mul(T[:, NPAIR + p : NPAIR + p + 1], tx, vc)

            def ray_mm(nd, T, gb, gw):
                # nd[s, 0:gw] = per-sample num, nd[s, gw:2gw] = den
                nc.tensor.matmul(
                    nd[:, 0:gw], ONE2[:], T[:, gb : gb + gw],
                    start=True, stop=True,
                )
                nc.tensor.matmul(
                    nd[:, gw : 2 * gw], ONE2[:],
                    T[:, NPAIR + gb : NPAIR + gb + gw],
                    start=True, stop=True,
                )

            # ---- slot stages ------------------------------------------
            xts = {}

            def st_dma(p):
                xt = xpool.tile([128, 2, KT * N], F32, tag="xt", name=f"xt{p}")
                xts[p] = xt
                for s in range(2):
                    nc.sync.dma_start(
                        xt[:, s, :].rearrange("q (b m) -> q b m", b=4),
                        xrs[p, s],
                    )

            def st_conv(p):
                xbt = xb[p % NXB]
                xsrc = xts[p].rearrange("q s (b r j) -> q s (b r) j", r=4, j=N)
                nc.scalar.copy(xbt[:, 0, :, 0:N], xsrc[:, 0])
                nc.gpsimd.tensor_copy(xbt[:, 1, :, 0:N], xsrc[:, 1])

            def st_gram(p):
                xbt = xb[p % NXB]
                ps = gram_slots.get(p)
                for k in range(KT):
                    for s in range(2):
                        r0 = s * B1
                        wk = xbt[:, s, k, :]
                        nc.tensor.matmul(
                            ps[r0 : r0 + B1, :],
                            wk,
                            wk,
                            start=(k == 0),
                            stop=(k == KT - 1),
                        )
                nc.scalar.copy(Amat[p][:], ps[:])

            def st_y(p):
                matvec(y_ps[p], 0, Amat[p], X0F[:, p : p + 1])
                nc.scalar.copy(Y[:, p : p + 1], y_ps[p][:])

            def st_g2(p):
                sq_copy(G2m[p], sq_mm(Amat[p], sq1_slots, p), nc.vector)

            def st_g4(p):
                sq_copy(G4m[p], sq_mm(G2m[p], sq1_slots, p), nc.vector)

            def st_z(p):
                g = pair_group[p]
                gw = GROUPS[g]
                j = p - group_base[g]
                m1 = mv1_ps[g]
                matvec(m1, 2 * gw + j, G4m[p], Y[:, p : p + 1])
                nc.scalar.copy(Z[:, p : p + 1], m1[:, 2 * gw + j : 2 * gw + j + 1])

            def st_x9(p):
                g = pair_group[p]
                m1 = mv1_ps[g]
                j = p - group_base[g]
                matvec(m1, j, G4m[p], Z[:, p : p + 1])
                nc.scalar.copy(X9[:, p : p + 1], m1[:, j : j + 1])

            def st_w1(p):
                g = pair_group[p]
                gw = GROUPS[g]
                j = p - group_base[g]
                m1 = mv1_ps[g]
                matvec(m1, gw + j, Amat[p], X9[:, p : p + 1])

            def st_t1(p):
                g = pair_group[p]
                ray_cols(p, mv1_ps[g], GROUPS[g], p - group_base[g], T1)

            def st_lam(g):
                gw = GROUPS[g]
                gb = group_base[g]
                nd = nd1_ps[g]
                ray_mm(nd, T1, gb, gw)
                lam = LAM[:, gb : gb + gw]
                rd = RDr[:, gb : gb + gw]
                nc.vector.reciprocal(rd, nd[:, gw : 2 * gw])
                nc.vector.tensor_mul(lam, nd[:, 0:gw], rd)
                psl = psl_ps[g]
                nc.tensor.matmul(psl, CMS[:], lam, start=True, stop=True)
                nc.scalar.copy(LAMV[:, gb : gb + gw], psl)

            def st_ball(g, half):
                gw = GROUPS[g]
                gb = group_base[g]
                m1 = mv1_ps[g]
                psl = psl_ps[g]
                for j in range(gw):
                    if j % 2 != half:
                        continue
                    pj = gb + j
                    scr = SCR[pj]
                    # u1 = S102*(w1 - lam*x9); x9 via its SBUF bf16 copy
                    # (a DVE op may read at most one PSUM operand)
                    t = scr[:, 2:3]
                    lv = LAMV[:, pj : pj + 1]
                    nc.vector.tensor_mul(t, lv, X9[:, pj : pj + 1])
                    d = scr[:, 3:4]
                    nc.vector.tensor_sub(d, m1[:, gw + j : gw + j + 1], t)
                    nc.vector.tensor_scalar(
                        U1[:, pj : pj + 1], d, S102, None, op0=ALU.mult
                    )
                    # Ball = A - lam*I (fp32); built on Pool from SBUF only
                    lamI = scr[:, 4 : 4 + PD]
                    nc.gpsimd.tensor_tensor(
                        lamI, DIAG[:], lv.broadcast_to([PG, PD]), op=ALU.mult
                    )
                    nc.gpsimd.tensor_sub(Ballm[pj][:], Amat[pj][:], lamI)

            def st_b2(g, half):
                for j in range(GROUPS[g]):
                    if j % 2 != half:
                        continue
                    pj = group_base[g] + j
                    sq_copy(B2m[pj], sq_mm(Ballm[pj], sq2_slots, j), nc.vector)

            def st_b4(g, half):
                for j in range(GROUPS[g]):
                    if j % 2 != half:
                        continue
                    pj = group_base[g] + j
                    sq_copy(B4m[pj], sq_mm(B2m[pj], sq2_slots, j), nc.vector)

            def st_b8(g, half):
                for j in range(GROUPS[g]):
                    if j % 2 != half:
                        continue
                    pj = group_base[g] + j
                    sq_copy(B8m[pj], sq_mm(B4m[pj], sq2_slots, j), nc.scalar)

            def st_u9(g, half):
                gw = GROUPS[g]
                gb = group_base[g]
                m2 = mv2_ps[g]
                for j in range(gw):
                    matvec(m2, j, B8m[gb + j], U1[:, gb + j : gb + j + 1])
                nc.scalar.copy(U9[:, gb : gb + gw], m2[:, 0:gw])

            def st_w2(g, half):
                gw = GROUPS[g]
                gb = group_base[g]
                m2 = mv2_ps[g]
                for j in range(gw):
                    matvec(m2, gw + j, Ballm[gb + j], U9[:, gb + j : gb + j + 1])
                for j in range(gw):
                    ray_cols(gb + j, m2, gw, j, T2)

            def st_pen(g):
                gw = GROUPS[g]
                gb = group_base[g]
                nd2 = nd2_ps[g]
                ray_mm(nd2, T2, gb, gw)
                s = slice(gb, gb + gw)
                nc.vector.reciprocal(RDr[:, s], nd2[:, gw : 2 * gw])
                nc.vector.tensor_mul(TMPr[:, s], nd2[:, 0:gw], RDr[:, s])
                nc.vector.tensor_add(SMr[:, s], TMPr[:, s], LAM[:, s])
                nc.vector.reciprocal(RSr[:, s], SMr[:, s])
                nc.vector.tensor_mul(RTr[:, s], TMPr[:, s], RSr[:, s])
                nc.vector.tensor_mul(PEN[:, s], RTr[:, s], RTr[:, s])

            # ---- slot schedule ----------------------------------------
            # pair p: dma@p, conv@p+1, gram@p+2, g2@p+3, g4@p+4, g8@p+5,
            # x9@p+6, w1+t1@p+7.  group g (last pair pe): lam@pe+7,
            # then ball/b2/b4/b8/u9/w2 split in j-halves, pen at the end.
            gsched = {}
            for g in range(len(GROUPS)):
                base = group_end[g] + (7 if group_end[g] < 12 else 6)
                seq = [
                    (st_lam, (g,)),
                    (st_ball, (g, 0)),
                    (st_ball, (g, 1)),
                    (st_b2, (g, 0)),
                    (st_b2, (g, 1)),
                    (st_b4, (g, 0)),
                    (st_b4, (g, 1)),
                    (st_b8, (g, 0)),
                    (st_b8, (g, 1)),
                    (st_u9, (g, 0)),
                    (st_w2, (g, 0)),
                    (st_pen, (g,)),
                ]
                # pack two mini-stages per slot (halves overlap pipelined)
                for i, (fn, args) in enumerate(seq[1:]):
                    gsched.setdefault(base + 1 + i // 2, []).append((fn, args))

            NSLOT = NPAIR + 16
            for s in range(NSLOT):
                # conversions first: they gate next slot's grams and their
                # DMA landed at the end of the previous slot.
                if 0 <= s - 1 < NPAIR:
                    st_conv(s - 1)
                if 0 <= s - 6 < NPAIR:
                    st_x9(s - 6)
                if 0 <= s - 5 < NPAIR:
                    st_z(s - 5)
                if 0 <= s - 4 < NPAIR:
                    st_g4(s - 4)
                if 0 <= s - 3 < NPAIR:
                    st_g2(s - 3)
                # pairs >= 11 run post-DMA: pull w1/T1/lam into the x9
                # slot (the in-slot PE stall no longer costs DMA cadence)
                if 0 <= s - 6 < NPAIR and s - 6 >= 12:
                    st_w1(s - 6)
                    st_t1(s - 6)
                    if s - 6 in group_end:
                        st_lam(pair_group[s - 6])
                if 0 <= s - 7 < NPAIR and s - 7 < 12:
                    st_w1(s - 7)
                    st_t1(s - 7)
                    if s - 7 in group_end:
                        st_lam(pair_group[s - 7])
                for fn, args in gsched.get(s, ()):
                    if fn is not None:
                        fn(*args)
                if 0 <= s - 2 < NPAIR:
                    st_gram(s - 2)
                    st_y(s - 2)
                if s < NPAIR:
                    st_dma(s)

            pen2 = pen.rearrange("(p two) -> two p", two=2)
            nc.sync.dma_start(pen2, PEN[:])
            if dbg is not None:
                nc.sync.dma_start(dbg[0], LAM[:])
                nc.sync.dma_start(dbg[1], TMPr[:])
                nc.sync.dma_start(dbg[2], SMr[:])
                DB = const.tile([PG, PD], F32, name="DB")
                nc.vector.tensor_copy(DB[:], Amat[0][:])
                nc.sync.dma_start(dbg2[0], DB[:])
                nc.sync.dma_start(dbg2[1, :, 0:NPAIR], LAMV[:])
                nc.sync.dma_start(dbg2[2], DIAG[:])
                DU = const.tile([PG, PD], F32, name="DU")
                nc.vector.tensor_copy(DU[:, 0:NPAIR], U1[:])
                nc.vector.tensor_copy(DU[:, NPAIR:2*NPAIR], U9[:])
                nc.vector.tensor_copy(DU[:, 2*NPAIR:3*NPAIR], X9[:])
                nc.sync.dma_start(dbg2[3], DU[:])


_NC_CACHE = {}


def build_nc(repeat=1):
    if repeat in _NC_CACHE:
        return _NC_CACHE[repeat]
    nc = bacc.Bacc("TRN2", target_bir_lowering=False, debug=False)
    x = nc.dram_tensor("x", [BS, C, N], F32, kind="ExternalInput")
    x0 = nc.dram_tensor("x0", [BS, N], F32, kind="ExternalInput")
    pen = nc.dram_tensor("pen", [BS], F32, kind="ExternalOutput")
    dbg = None
    if os.environ.get("KERNEL_DEBUG"):
        dbgt = nc.dram_tensor("dbg", [3, 2, NPAIR], F32, kind="ExternalOutput")
        dbg = dbgt.ap()
        dbgt2 = nc.dram_tensor("dbg2", [4, PG, PD], F32, kind="ExternalOutput")
        globals()["_dbg2"] = dbgt2
    with tile.TileContext(nc) as tc:
        dbg2 = globals()["_dbg2"].ap() if dbg is not None else None
        _emit(tc, x.ap(), x0.ap(), pen.ap(), repeat=repeat, dbg=dbg, dbg2=dbg2)
    nc.compile()
    _NC_CACHE[repeat] = nc
    return nc


LAST_RESULTS = None


def kernel(x, x0):
    global LAST_RESULTS
    x = np.ascontiguousarray(np.asarray(x, dtype=np.float32).reshape(B, C, N))
    x0 = np.ascontiguousarray(np.asarray(x0, dtype=np.float32).reshape(B, N))
    nc = build_nc()
    in_maps = [
        {"x": x[i * BS : (i + 1) * BS], "x0": x0[i * BS : (i + 1) * BS]}
        for i in range(NCORES)
    ]
    trace = bool(int(os.environ.get("KERNEL_TRACE", "0")))
    res = run_bass_kernel_spmd(nc, in_maps, list(range(NCORES)), trace=trace)
    LAST_RESULTS = res
    pens = np.concatenate([r["pen"].reshape(-1) for r in res.results])
    return np.float32(pens.sum(dtype=np.float64) / B)
